# revision 14
# baseline (speedup 1.0000x reference)
"""Trainium2 Bass kernel for nn_CausalSelfAttention_16149077032974.

Full inputs in, full outputs out. Sharding: data-parallel over B (2 groups of
4 cores), tensor-parallel over heads within a group (4 heads/core). Each core
runs the whole per-head pipeline (QKVG projections, RoPE, QK-RMSNorm, causal
SDPA, output RMSNorm, silu gate, c_proj partial); the c_proj all-reduce is done
on the host while gathering (the partial sums are exact in f32).

Per-core kernel layout choices:
 - x is passed pre-transposed (H, T) so every projection contracts over H on
   the partition axis with no on-chip transpose of x.
 - Q/K are projected W-stationary so they come out of the PE directly in
   (head_dim, token) layout -- no PE transposes.  RoPE/RMSNorm run in that
   layout: the per-token sumsq is an all-ones-stationary matmul whose output
   is replicated across partitions; the rotation reads the PSUM accumulator
   at a +-64 partition offset.  k-outer sub-group loops start matmuls as
   soon as the first x/W k-tiles land (no DMA head).
 - Scores are built transposed, S^T[k, q] = K̂·Q̂^T, one 128-row k-tile at a
   time; softmax needs no max subtraction (|S|·scale <= ~11.3 since q,k are
   RMS-normalized), so E = exp(scale·S^T) directly, with a ones-column
   appended to V to accumulate the denominators inside the same PV matmul.
 - Normalization (softmax denom + output RMSNorm) collapses into one
   per-token scalar 1/sqrt(sumsq(y_un)/HD + eps·s²) applied after PV.
 - q_gamma*k_gamma is folded into K's RoPE tables; o_gamma into Wo (host).
"""

import numpy as np
import ml_dtypes
from contextlib import ExitStack

import concourse.bass as bass
import concourse.tile as tile
from concourse import bacc, mybir
from concourse import bass_utils
from concourse.bass import ts
from concourse.masks import make_identity

AL = None  # set below

BF16 = ml_dtypes.bfloat16
AL = mybir.AluOpType
F32 = mybir.dt.float32
BF = mybir.dt.bfloat16

B, T, H = 2, 2048, 2048
NH, HD = 16, 128
EPS = 1e-5
ROPE_BASE = 10000.0
NHL = 4          # heads per core
F = NHL * HD     # local feature width (512)
TT = T // 128    # 16 token tiles
CH = T // 512    # 4 query chunks
SCALE = 1.0 / float(np.sqrt(HD))

_CACHE = {}


def _build_nc():
    nc = bacc.Bacc("TRN2", target_bir_lowering=False, debug=False)

    xT_d = nc.dram_tensor("xT", [H, T], BF, kind="ExternalInput")
    wqT_d = nc.dram_tensor("wqT", [H, F], BF, kind="ExternalInput")
    wkT_d = nc.dram_tensor("wkT", [H, F], BF, kind="ExternalInput")
    wvT_d = nc.dram_tensor("wvT", [H, F], BF, kind="ExternalInput")
    wgT_d = nc.dram_tensor("wgT", [H, F], BF, kind="ExternalInput")
    woT_d = nc.dram_tensor("woT", [F, H], BF, kind="ExternalInput")
    # RoPE tables in (head_dim, token) layout; sin sign-folded, K gamma-folded
    cosq_d = nc.dram_tensor("cosqT", [HD, T], BF, kind="ExternalInput")
    sinq_d = nc.dram_tensor("sinqT", [HD, T], BF, kind="ExternalInput")
    cosk_d = nc.dram_tensor("coskT", [HD, T], BF, kind="ExternalInput")
    sink_d = nc.dram_tensor("sinkT", [HD, T], BF, kind="ExternalInput")
    tri_d = nc.dram_tensor("tri", [128, 128], BF, kind="ExternalInput")
    out_d = nc.dram_tensor("out", [T, H], BF, kind="ExternalOutput")

    with tile.TileContext(nc) as tc:
        with ExitStack() as outer:
            # ---- persistent pools (live across all phases) ----
            consts = outer.enter_context(tc.tile_pool(name="consts", bufs=1))
            qkt = outer.enter_context(tc.tile_pool(name="qkt", bufs=1))
            vpool = outer.enter_context(tc.tile_pool(name="vpool", bufs=1))
            gpool = outer.enter_context(tc.tile_pool(name="gpool", bufs=1))
            spool = outer.enter_context(tc.tile_pool(name="spool", bufs=1))

            id128 = consts.tile([128, 128], BF, tag="id")
            make_identity(nc, id128[:, :])
            tri = consts.tile([128, 128], BF, tag="tri")
            eps_t = consts.tile([128, 1], F32, tag="eps")
            nc.vector.memset(eps_t[:, :], EPS)
            ones128 = consts.tile([128, 128], BF, tag="ones")
            nc.vector.memset(ones128[:, :], 1.0)

            QT = [qkt.tile([128, T], BF, tag=f"qt{h}", name=f"QT{h}") for h in range(NHL)]
            KT = [qkt.tile([128, T], BF, tag=f"kt{h}", name=f"KT{h}") for h in range(NHL)]
            # V with a ones column appended per (ktile, head): [...,128]=1.0
            vaug = vpool.tile([128, TT, NHL, 132], BF, tag="vaug")
            nc.vector.memset(vaug[:, :, :, 128:129], 1.0)
            gs = gpool.tile([128, TT, F], BF, tag="gs")       # silu(gate)
            m_all = spool.tile([128, TT, NHL], F32, tag="mall")
            stok = spool.tile([128, TT, NHL], F32, tag="stok")

            # ================= phase A: projections =================
            # Q/K are projected W-stationary so they come out directly in
            # (head_dim, token) layout -- no PE transposes.  k-outer loops
            # over sub-groups of <=6 PSUM accumulators let the very first
            # matmuls start as soon as the first x/W k-tiles land (no
            # 28us DMA head).  RMSNorm stats use an all-ones stationary
            # matmul whose output is the per-token sumsq REPLICATED across
            # all 128 partitions (solves the partition-broadcast problem);
            # the RoPE rotation reads the PSUM accumulator at a +-64
            # partition offset (legal: in0 is PSUM).
            with ExitStack() as pa:
                xpool = pa.enter_context(tc.tile_pool(name="xpool", bufs=1))
                wpool = pa.enter_context(tc.tile_pool(name="wpool", bufs=20))
                pcs = pa.enter_context(tc.tile_pool(name="pcs", bufs=2))
                psq = pa.enter_context(tc.tile_pool(name="psq", bufs=6))
                psd = pa.enter_context(tc.tile_pool(name="psd", bufs=2))
                prstd = pa.enter_context(tc.tile_pool(name="prstd", bufs=2))
                prc = pa.enter_context(tc.tile_pool(name="prc", bufs=6))
                psA = pa.enter_context(tc.tile_pool(name="psA", bufs=6, space="PSUM"))
                psM = pa.enter_context(tc.tile_pool(name="psM", bufs=2, space="PSUM"))

                xT = xpool.tile([128, TT, T], BF, tag="xT")
                xr = xT_d.ap().rearrange("(k p) t -> p k t", p=128)

                def load_w16(w_d):
                    wr = w_d.ap().rearrange("(k p) f -> p k f", p=128)
                    parts = []
                    for k in range(TT):
                        wt = wpool.tile([128, F], BF, tag="w", name=f"w_{w_d.name}_{k}")
                        nc.sync.dma_start(out=wt[:, :], in_=wr[:, k, :])
                        parts.append(wt)
                    return parts

                # DMA order (HWDGE FIFO): per-k [wq, x-first-half] pairs so Q
                # SG0 streams; then Q tables, x second halves, wk, K tables
                # (the K tables reuse the Q tables' two slots, so they queue
                # after everything needed earlier).
                wq16 = []
                wr_q = wqT_d.ap().rearrange("(k p) f -> p k f", p=128)
                for k in range(TT):
                    wt = wpool.tile([128, F], BF, tag="w", name=f"w_q_{k}")
                    nc.sync.dma_start(out=wt[:, :], in_=wr_q[:, k, :])
                    wq16.append(wt)
                    nc.sync.dma_start(out=xT[:, k, 0:1024], in_=xr[:, k, 0:1024])
                cosq = pcs.tile([128, T], BF, tag="tbl", name="cosq")
                nc.sync.dma_start(out=cosq[:, :], in_=cosq_d.ap())
                sinq = pcs.tile([128, T], BF, tag="tbl", name="sinq")
                nc.sync.dma_start(out=sinq[:, :], in_=sinq_d.ap())
                nc.sync.dma_start(out=tri[:, :], in_=tri_d.ap())
                for k in range(TT):
                    nc.sync.dma_start(out=xT[:, k, 1024:2048], in_=xr[:, k, 1024:2048])
                wk16 = load_w16(wkT_d)
                cosk = pcs.tile([128, T], BF, tag="tbl", name="cosk")
                nc.sync.dma_start(out=cosk[:, :], in_=cosk_d.ap())
                sink = pcs.tile([128, T], BF, tag="tbl", name="sink")
                nc.sync.dma_start(out=sink[:, :], in_=sink_d.ap())

                # sub-groups of (head, chunk) units; SG0 only touches token
                # chunks 0-1 so it can start on the first x half-rows.
                SGS = [
                    [(0, 0), (1, 0), (2, 0), (3, 0), (0, 1), (1, 1)],
                    [(2, 1), (3, 1), (0, 2), (1, 2), (2, 2), (3, 2)],
                    [(0, 3), (1, 3), (2, 3), (3, 3)],
                ]

                def qk_phase(w16, targets, cosT, sinT):
                    def evict_early(accs, sg):
                        out = []
                        for a, (h, c) in zip(accs, sg):
                            sq = psq.tile([128, F], BF, tag="sq")
                            nc.scalar.activation(
                                sq[:, :], a[:, :],
                                mybir.ActivationFunctionType.Square,
                            )
                            rc = prc.tile([128, F], BF, tag="rc")
                            nc.vector.tensor_mul(rc[:, :], a[:, :], cosT[:, ts(c, 512)])
                            rot = prc.tile([128, F], BF, tag="rot")
                            nc.vector.tensor_mul(
                                rot[0:64, :], a[64:128, :], sinT[0:64, ts(c, 512)]
                            )
                            nc.vector.tensor_mul(
                                rot[64:128, :], a[0:64, :], sinT[64:128, ts(c, 512)]
                            )
                            out.append((sq, rc, rot))
                        return out

                    def evict_late(unit, sg, early):
                        sq, rc, rot = early[unit]
                        h, c = sg[unit]
                        msq = psM.tile([128, F], F32, tag="msq")
                        nc.tensor.matmul(
                            msq[:, :], ones128[:, :], sq[:, :], start=True, stop=True
                        )
                        sd = psd.tile([128, F], F32, tag="sd")
                        nc.scalar.activation(
                            sd[:, :], msq[:, :],
                            mybir.ActivationFunctionType.Sqrt,
                            bias=eps_t[:, :], scale=1.0 / HD,
                        )
                        rstd = prstd.tile([128, F], F32, tag="rstd")
                        nc.vector.reciprocal(rstd[:, :], sd[:, :])
                        summ = prc.tile([128, F], BF, tag="summ", bufs=2)
                        nc.vector.tensor_add(summ[:, :], rc[:, :], rot[:, :])
                        nc.vector.tensor_mul(
                            targets[h][:, ts(c, 512)], summ[:, :], rstd[:, :]
                        )

                    pend = None
                    for sg in SGS:
                        accs = [
                            psA.tile([128, F], F32, tag="acc", name="acc")
                            for i in range(len(sg))
                        ]
                        for k in range(TT):
                            for a, (h, c) in zip(accs, sg):
                                nc.tensor.matmul(
                                    a[:, :], w16[k][:, ts(h, 128)],
                                    xT[:, k, ts(c, 512)],
                                    start=(k == 0), stop=(k == TT - 1),
                                )
                            # previous SG's late evictions interleave with
                            # this SG's k-loop (2 per k keeps the 2 msq
                            # PSUM banks rotating without stalling the PE)
                            if pend is not None and k >= 1 and 2 * (k - 1) < len(pend[1]):
                                for u in (2 * (k - 1), 2 * (k - 1) + 1):
                                    if u < len(pend[1]):
                                        evict_late(u, pend[1], pend[0])
                        if pend is not None and 2 * (TT - 1) < len(pend[1]):
                            for u in range(2 * (TT - 1), len(pend[1])):
                                evict_late(u, pend[1], pend[0])
                        early = evict_early(accs, sg)
                        pend = (early, sg)
                    for u in range(len(pend[1])):
                        evict_late(u, pend[1], pend[0])

                qk_phase(wq16, QT, cosq, sinq)
                qk_phase(wk16, KT, cosk, sink)

                # -- V / G: x-stationary, token-partition output (unchanged
                #    math, per-k weight tiles) --
                def matmul_proj(acc, w16, t):
                    for k in range(TT):
                        nc.tensor.matmul(
                            acc[:, :], xT[:, k, ts(t, 128)], w16[k][:, :],
                            start=(k == 0), stop=(k == TT - 1),
                        )

                wv16 = load_w16(wvT_d)
                for t in range(TT):
                    acc = psA.tile([128, F], F32, tag="acc")
                    matmul_proj(acc, wv16, t)
                    nc.scalar.copy(
                        vaug[:, t, :, 0:128],
                        acc[:, :].rearrange("p (h d) -> p h d", h=NHL),
                    )

                wg16 = load_w16(wgT_d)
                for t in range(TT):
                    acc = psA.tile([128, F], F32, tag="acc")
                    matmul_proj(acc, wg16, t)
                    nc.scalar.activation(
                        gs[:, t, :], acc[:, :], mybir.ActivationFunctionType.Silu
                    )

            # yun / woT pools are entered after phase A's pools release so
            # their SBUF space overlaps the (freed) x / weight staging area
            ypool = outer.enter_context(tc.tile_pool(name="ypool", bufs=1))
            wopool = outer.enter_context(tc.tile_pool(name="wopool", bufs=1))
            yun = ypool.tile([128, TT, NHL, HD], BF, tag="yun")  # unnormalized y
            woT = wopool.tile([128, 4, H], BF, tag="woT")
            nc.sync.dma_start(
                out=woT[:, :, :],
                in_=woT_d.ap().rearrange("(k p) n -> p k n", p=128),
            )

            # ================= phase B: SDPA =================
            with ExitStack() as pb:
                pE = pb.enter_context(tc.tile_pool(name="pE", bufs=21))
                pys = pb.enter_context(tc.tile_pool(name="pys", bufs=8))
                pyscr = pb.enter_context(tc.tile_pool(name="pyscr", bufs=2))
                pnw = pb.enter_context(tc.tile_pool(name="pnw", bufs=2))
                psS = pb.enter_context(tc.tile_pool(name="psS", bufs=3, space="PSUM"))
                psY = pb.enter_context(tc.tile_pool(name="psY", bufs=2, space="PSUM"))

                def s_block(c, h):
                    # S^T tiles, two k-tiles per 2-bank PSUM tile so the exp
                    # runs as one wide ACTIVATE (halves the per-op overhead)
                    elist = []
                    for kt0 in range(0, 4 * c + 4, 2):
                        e2 = pE.tile([128, 2, 512], BF, tag="e")
                        s2 = psS.tile([128, 2, 512], F32, tag="s")
                        diag = kt0 >= 4 * c
                        for j in range(2):
                            kt = kt0 + j
                            lo = 0 if kt < 4 * c else 128 * (kt - 4 * c)
                            nc.tensor.matmul(
                                s2[:, j, lo:512], KT[h][:, ts(kt, 128)],
                                QT[h][:, 512 * c + lo:512 * c + 512],
                                start=True, stop=True,
                            )
                        if not diag:
                            nc.scalar.activation(
                                e2[:, :, :], s2[:, :, :],
                                mybir.ActivationFunctionType.Exp, scale=SCALE,
                            )
                        else:
                            for j in range(2):
                                d = kt0 + j - 4 * c
                                lo = 128 * d
                                nc.scalar.activation(
                                    e2[:, j, lo:512], s2[:, j, lo:512],
                                    mybir.ActivationFunctionType.Exp, scale=SCALE,
                                )
                                nc.vector.tensor_mul(
                                    e2[:, j, lo:lo + 128], e2[:, j, lo:lo + 128],
                                    tri[:, :],
                                )
                        elist.append(e2)
                    return elist

                def pv_block(c, h, elist):
                    # PV + denominators, two query tiles per PSUM tile so the
                    # stats run batched on the DVE
                    for p in range(2):
                        y_ps = psY.tile([128, 2, 132], F32, tag="y")
                        for qp in range(2):
                            qt = 2 * p + qp
                            t = 4 * c + qt
                            for kt in range(t + 1):
                                nc.tensor.matmul(
                                    y_ps[:, qp, 0:129],
                                    elist[kt // 2][:, kt % 2, ts(qt, 128)],
                                    vaug[:, kt, h, 0:129],
                                    start=(kt == 0), stop=(kt == t),
                                )
                        tpair = 4 * c + 2 * p
                        # evict unnormalized y, then m = sumsq/HD + eps*s^2
                        nc.vector.tensor_copy(
                            yun[:, tpair:tpair + 2, h, :], y_ps[:, :, 0:128]
                        )
                        scol = pys.tile([128, 2], F32, tag="scol")
                        nc.vector.tensor_copy(scol[:, :], y_ps[:, :, 128])
                        s2e = pys.tile([128, 2], F32, tag="s2e")
                        nc.vector.tensor_mul(s2e[:, :], scol[:, :], scol[:, :])
                        ysq = pyscr.tile([128, 2, HD], BF, tag="ysq")
                        nc.vector.tensor_mul(
                            ysq[:, :, :], yun[:, tpair:tpair + 2, h, :],
                            yun[:, tpair:tpair + 2, h, :],
                        )
                        ss = pys.tile([128, 2], F32, tag="ss")
                        nc.vector.tensor_reduce(
                            out=ss[:, :], in_=ysq[:, :, :],
                            axis=mybir.AxisListType.X, op=mybir.AluOpType.add,
                        )
                        sse = pys.tile([128, 2], F32, tag="sse")
                        nc.vector.tensor_scalar(
                            out=sse[:, :], in0=ss[:, :], scalar1=1.0 / HD,
                            scalar2=None, op0=mybir.AluOpType.mult,
                        )
                        s2es = pys.tile([128, 2], F32, tag="s2es")
                        nc.vector.tensor_scalar(
                            out=s2es[:, :], in0=s2e[:, :], scalar1=EPS,
                            scalar2=None, op0=mybir.AluOpType.mult,
                        )
                        nc.vector.tensor_add(
                            m_all[:, tpair:tpair + 2, h], s2es[:, :], sse[:, :],
                        )

                def stok_chunk(c):
                    # stok = 1/sqrt(m) via bit-trick + 2 Newton steps, all on
                    # the DVE so the (ACT-bound) exp stream is untouched
                    y = stok[:, 4 * c:4 * c + 4, :]
                    mm = m_all[:, 4 * c:4 * c + 4, :]
                    hm = pnw.tile([128, 4, NHL], F32, tag="hm")
                    aa = pnw.tile([128, 4, NHL], F32, tag="aa")
                    cc = pnw.tile([128, 4, NHL], F32, tag="cc")
                    yi = y.bitcast(mybir.dt.int32)
                    nc.vector.tensor_scalar(
                        out=yi, in0=mm.bitcast(mybir.dt.int32), scalar1=1,
                        scalar2=None, op0=AL.logical_shift_right)
                    nc.vector.tensor_scalar(
                        out=yi, in0=yi, scalar1=0x5F3759DF, scalar2=-1,
                        op0=AL.subtract, op1=AL.mult)
                    nc.vector.tensor_scalar(
                        out=hm[:, :, :], in0=mm, scalar1=0.5, scalar2=None,
                        op0=AL.mult)
                    for _ in range(2):
                        nc.vector.tensor_mul(aa[:, :, :], y, y)
                        nc.vector.tensor_mul(aa[:, :, :], aa[:, :, :], hm[:, :, :])
                        nc.vector.tensor_scalar(
                            out=cc[:, :, :], in0=aa[:, :, :], scalar1=-1.0,
                            scalar2=1.5, op0=AL.mult, op1=AL.add)
                        nc.vector.tensor_mul(y, y, cc[:, :, :])

                units = [(c, h) for c in range(CH) for h in range(NHL)]
                prev = None
                for (c, h) in units:
                    el = s_block(c, h)
                    if prev is not None:
                        pv_block(*prev)
                        if prev[1] == NHL - 1:
                            stok_chunk(prev[0])
                    prev = (c, h, el)
                pv_block(*prev)
                stok_chunk(prev[0])

            # ================= phase C: normalize, gate, c_proj =================
            with ExitStack() as pc:
                pyg = pc.enter_context(tc.tile_pool(name="pyg", bufs=10))
                pygT = pc.enter_context(tc.tile_pool(name="pygT", bufs=16))
                pout = pc.enter_context(tc.tile_pool(name="pout", bufs=6))
                psTc = pc.enter_context(tc.tile_pool(name="psTc", bufs=4, space="PSUM"))
                psO = pc.enter_context(tc.tile_pool(name="psO", bufs=4, space="PSUM"))

                def ygm(t):
                    out = []
                    for hh in range(NHL):
                        t1 = pyg.tile([128, HD], BF, tag="t1")
                        nc.vector.tensor_mul(
                            t1[:, :], yun[:, t, hh, :], gs[:, t, ts(hh, 128)]
                        )
                        yg = pyg.tile([128, HD], BF, tag="yg", name=f"yg_{t}_{hh}")
                        if hh % 2 == 0:
                            nc.scalar.mul(yg[:, :], t1[:, :], stok[:, t, hh:hh + 1])
                        else:
                            nc.vector.tensor_scalar_mul(
                                yg[:, :], t1[:, :], stok[:, t, hh:hh + 1]
                            )
                        out.append(yg)
                    return out

                def emit_T(ygl):
                    out = []
                    for hh in range(NHL):
                        tp = psTc.tile([128, 128], BF, tag="tp")
                        nc.tensor.transpose(tp[:, :], ygl[hh][:, :], id128[:, :])
                        yt = pygT.tile([128, 128], BF, tag="yt")
                        if hh % 2 == 0:
                            nc.scalar.copy(yt[:, :], tp[:, :])
                        else:
                            nc.vector.tensor_copy(yt[:, :], tp[:, :])
                        out.append(yt)
                    return out

                pendT = [emit_T(ygm(0)), emit_T(ygm(1))]
                for t in range(TT):
                    ygT = pendT.pop(0)
                    if t + 2 < TT:
                        pendT.append(emit_T(ygm(t + 2)))
                    for n in range(4):
                        o_ps = psO.tile([128, 512], F32, tag="o")
                        for f in range(4):
                            nc.tensor.matmul(
                                o_ps[:, :], ygT[f][:, :], woT[:, f, ts(n, 512)],
                                start=(f == 0), stop=(f == 3),
                            )
                        o_sb = pout.tile([128, 512], BF, tag="osb")
                        if n % 2 == 0:
                            nc.vector.tensor_copy(o_sb[:, :], o_ps[:, :])
                        else:
                            nc.scalar.copy(o_sb[:, :], o_ps[:, :])
                        nc.sync.dma_start(
                            out=out_d.ap()[ts(t, 128), ts(n, 512)], in_=o_sb[:, :]
                        )

    nc.compile()
    return nc


def _rope_tables():
    inv_freq = 1.0 / (ROPE_BASE ** (np.arange(0, HD, 2, dtype=np.float32) / HD))
    t = np.arange(T, dtype=np.float32)
    freqs = t[:, None] * inv_freq[None, :]
    emb = np.concatenate([freqs, freqs], axis=-1)
    return np.cos(emb).astype(np.float32), np.sin(emb).astype(np.float32)


def _host_prep(x, Wq, Wk, Wv, Wg, Wo, q_gamma, k_gamma, o_gamma):
    x = np.asarray(x, dtype=np.float32)
    Wq = np.asarray(Wq, dtype=np.float32)
    Wk = np.asarray(Wk, dtype=np.float32)
    Wv = np.asarray(Wv, dtype=np.float32)
    Wg = np.asarray(Wg, dtype=np.float32)
    Wo = np.asarray(Wo, dtype=np.float32)
    q_gamma = np.asarray(q_gamma, dtype=np.float32)
    k_gamma = np.asarray(k_gamma, dtype=np.float32)
    o_gamma = np.asarray(o_gamma, dtype=np.float32)

    cos, sin = _rope_tables()
    sinm_f = np.concatenate([-sin[:, :64], sin[:, 64:]], axis=1)
    # (head_dim, token) layout tables; q_gamma*k_gamma folds into K's tables
    gqk = (q_gamma * k_gamma).astype(np.float32)
    cosqT = np.ascontiguousarray(cos.T).astype(BF16)
    sinqT = np.ascontiguousarray(sinm_f.T).astype(BF16)
    coskT = np.ascontiguousarray((cos * gqk[None, :]).T).astype(BF16)
    sinkT = np.ascontiguousarray((sinm_f * gqk[None, :]).T).astype(BF16)
    tri = (np.arange(128)[None, :] >= np.arange(128)[:, None]).astype(BF16)

    xTb = [np.ascontiguousarray(x[b].T).astype(BF16) for b in range(B)]
    per_group = []
    for g in range(4):
        hs = slice(g * F, (g + 1) * F)
        wo_scaled = Wo[:, hs] * np.tile(o_gamma, NHL)[None, :]
        per_group.append({
            "wqT": np.ascontiguousarray(Wq[hs].T).astype(BF16),
            "wkT": np.ascontiguousarray(Wk[hs].T).astype(BF16),
            "wvT": np.ascontiguousarray(Wv[hs].T).astype(BF16),
            "wgT": np.ascontiguousarray(Wg[hs].T).astype(BF16),
            "woT": np.ascontiguousarray(wo_scaled.T).astype(BF16),
        })

    in_maps = []
    for c in range(8):
        b, g = c // 4, c % 4
        m = {"xT": xTb[b], "cosqT": cosqT, "sinqT": sinqT, "coskT": coskT,
             "sinkT": sinkT, "tri": tri}
        m.update(per_group[g])
        in_maps.append(m)
    return in_maps


def kernel(x, Wq, Wk, Wv, Wg, Wo, q_gamma, k_gamma, o_gamma):
    if "nc" not in _CACHE:
        _CACHE["nc"] = _build_nc()
    nc = _CACHE["nc"]
    in_maps = _host_prep(x, Wq, Wk, Wv, Wg, Wo, q_gamma, k_gamma, o_gamma)
    res = bass_utils.run_bass_kernel_spmd(nc, in_maps, core_ids=list(range(8)))
    out = np.empty((B, T, H), dtype=np.float32)
    for b in range(B):
        acc = res.results[4 * b]["out"].astype(np.float32)
        for g in range(1, 4):
            acc = acc + res.results[4 * b + g]["out"].astype(np.float32)
        out[b] = acc
    return out



# revision 18
# speedup vs baseline: 1.3308x; 1.3308x over previous
"""Trainium2 Bass kernel for nn_CausalSelfAttention_16149077032974.

Full inputs in, full outputs out. Sharding: data-parallel over B (2 groups of
4 cores), tensor-parallel over heads within a group (4 heads/core). Each core
runs the whole per-head pipeline (QKVG projections, RoPE, QK-RMSNorm, causal
SDPA, output RMSNorm, silu gate, c_proj partial); the c_proj all-reduce is done
on the host while gathering (the partial sums are exact in f32).

Per-core kernel layout choices:
 - x is passed pre-transposed (H, T) so every projection contracts over H on
   the partition axis with no on-chip transpose of x.
 - Q/K are projected W-stationary so they come out of the PE directly in
   (head_dim, token) layout -- no PE transposes.  RoPE/RMSNorm run in that
   layout: the per-token sumsq is an all-ones-stationary matmul whose output
   is replicated across partitions; the rotation reads the PSUM accumulator
   at a +-64 partition offset.  k-outer sub-group loops start matmuls as
   soon as the first x/W k-tiles land (no DMA head).
 - Scores are built transposed, S^T[k, q] = K̂·Q̂^T, one 128-row k-tile at a
   time; softmax needs no max subtraction (|S|·scale <= ~11.3 since q,k are
   RMS-normalized), so E = exp(scale·S^T) directly, with a ones-column
   appended to V to accumulate the denominators inside the same PV matmul.
 - Normalization (softmax denom + output RMSNorm) collapses into one
   per-token scalar 1/sqrt(sumsq(y_un)/HD + eps·s²) applied after PV.
 - q_gamma*k_gamma is folded into K's RoPE tables; o_gamma into Wo (host).
"""

import numpy as np
import ml_dtypes
from contextlib import ExitStack

import concourse.bass as bass
import concourse.tile as tile
from concourse import bacc, mybir
from concourse import bass_utils
from concourse.bass import ts
from concourse.masks import make_identity

AL = None  # set below

BF16 = ml_dtypes.bfloat16
AL = mybir.AluOpType
F32 = mybir.dt.float32
BF = mybir.dt.bfloat16

B, T, H = 2, 2048, 2048
NH, HD = 16, 128
EPS = 1e-5
ROPE_BASE = 10000.0
NHL = 4          # heads per core
F = NHL * HD     # local feature width (512)
TT = T // 128    # 16 token tiles
CH = T // 512    # 4 query chunks
SCALE = 1.0 / float(np.sqrt(HD))

_CACHE = {}


def _build_nc():
    nc = bacc.Bacc("TRN2", target_bir_lowering=False, debug=False)

    xT_d = nc.dram_tensor("xT", [H, T], BF, kind="ExternalInput")
    wqT_d = nc.dram_tensor("wqT", [H, F], BF, kind="ExternalInput")
    wkT_d = nc.dram_tensor("wkT", [H, F], BF, kind="ExternalInput")
    wvT_d = nc.dram_tensor("wvT", [H, F], BF, kind="ExternalInput")
    wgT_d = nc.dram_tensor("wgT", [H, F], BF, kind="ExternalInput")
    woT_d = nc.dram_tensor("woT", [F, H], BF, kind="ExternalInput")
    # RoPE tables in (head_dim, token) layout; sin sign-folded, K gamma-folded
    cosq_d = nc.dram_tensor("cosqT", [HD, T], BF, kind="ExternalInput")
    sinq_d = nc.dram_tensor("sinqT", [HD, T], BF, kind="ExternalInput")
    cosk_d = nc.dram_tensor("coskT", [HD, T], BF, kind="ExternalInput")
    sink_d = nc.dram_tensor("sinkT", [HD, T], BF, kind="ExternalInput")
    tri_d = nc.dram_tensor("tri", [128, 128], BF, kind="ExternalInput")
    out_d = nc.dram_tensor("out", [T, H], BF, kind="ExternalOutput")

    with tile.TileContext(nc) as tc:
        with ExitStack() as outer:
            # ---- persistent pools (live across all phases) ----
            consts = outer.enter_context(tc.tile_pool(name="consts", bufs=1))
            qkt = outer.enter_context(tc.tile_pool(name="qkt", bufs=1))
            vpool = outer.enter_context(tc.tile_pool(name="vpool", bufs=1))
            gpool = outer.enter_context(tc.tile_pool(name="gpool", bufs=1))
            spool = outer.enter_context(tc.tile_pool(name="spool", bufs=1))

            id128 = consts.tile([128, 128], BF, tag="id")
            make_identity(nc, id128[:, :])
            tri = consts.tile([128, 128], BF, tag="tri")
            eps_t = consts.tile([128, 1], F32, tag="eps")
            nc.vector.memset(eps_t[:, :], EPS)
            ones128 = consts.tile([128, 128], BF, tag="ones")
            nc.vector.memset(ones128[:, :], 1.0)

            QT = [qkt.tile([128, T], BF, tag=f"qt{h}", name=f"QT{h}") for h in range(NHL)]
            KT = [qkt.tile([128, T], BF, tag=f"kt{h}", name=f"KT{h}") for h in range(NHL)]
            # V with a ones column appended per (ktile, head): [...,128]=1.0
            vaug = vpool.tile([128, TT, NHL, 132], BF, tag="vaug")
            nc.vector.memset(vaug[:, :, :, 128:129], 1.0)
            gs = gpool.tile([128, TT, F], BF, tag="gs")       # silu(gate)
            m_all = spool.tile([128, TT, NHL], F32, tag="mall")
            stok = spool.tile([128, TT, NHL], F32, tag="stok")

            # ================= phase A: projections =================
            # Q/K are projected W-stationary so they come out directly in
            # (head_dim, token) layout -- no PE transposes.  k-outer loops
            # over sub-groups of <=6 PSUM accumulators let the very first
            # matmuls start as soon as the first x/W k-tiles land (no
            # 28us DMA head).  RMSNorm stats use an all-ones stationary
            # matmul whose output is the per-token sumsq REPLICATED across
            # all 128 partitions (solves the partition-broadcast problem);
            # the RoPE rotation reads the PSUM accumulator at a +-64
            # partition offset (legal: in0 is PSUM).
            with ExitStack() as pa:
                xpool = pa.enter_context(tc.tile_pool(name="xpool", bufs=1))
                wpool = pa.enter_context(tc.tile_pool(name="wpool", bufs=32))
                pcs = pa.enter_context(tc.tile_pool(name="pcs", bufs=2))
                psq = pa.enter_context(tc.tile_pool(name="psq", bufs=6))
                prstd = pa.enter_context(tc.tile_pool(name="prstd", bufs=2))
                prc = pa.enter_context(tc.tile_pool(name="prc", bufs=6))
                psA = pa.enter_context(tc.tile_pool(name="psA", bufs=6, space="PSUM"))
                psM = pa.enter_context(tc.tile_pool(name="psM", bufs=2, space="PSUM"))

                xT = xpool.tile([128, TT, T], BF, tag="xT")
                xr = xT_d.ap().rearrange("(k p) t -> p k t", p=128)

                def load_w16(w_d):
                    wr = w_d.ap().rearrange("(k p) f -> p k f", p=128)
                    parts = []
                    for k in range(TT):
                        wt = wpool.tile([128, F], BF, tag="w", name=f"w_{w_d.name}_{k}")
                        nc.sync.dma_start(out=wt[:, :], in_=wr[:, k, :])
                        parts.append(wt)
                    return parts

                # DMA order (HWDGE FIFO): per-k [wq, x-first-half] pairs so Q
                # SG0 streams; then Q tables, x second halves, wk, K tables
                # (the K tables reuse the Q tables' two slots, so they queue
                # after everything needed earlier).
                wq16 = []
                wr_q = wqT_d.ap().rearrange("(k p) f -> p k f", p=128)
                for k in range(TT):
                    wt = wpool.tile([128, F], BF, tag="w", name=f"w_q_{k}")
                    nc.sync.dma_start(out=wt[:, :], in_=wr_q[:, k, :])
                    wq16.append(wt)
                    nc.sync.dma_start(out=xT[:, k, 0:1024], in_=xr[:, k, 0:1024])
                cosq = pcs.tile([128, T], BF, tag="tbl", name="cosq")
                nc.sync.dma_start(out=cosq[:, :], in_=cosq_d.ap())
                sinq = pcs.tile([128, T], BF, tag="tbl", name="sinq")
                nc.sync.dma_start(out=sinq[:, :], in_=sinq_d.ap())
                nc.sync.dma_start(out=tri[:, :], in_=tri_d.ap())
                for k in range(TT):
                    nc.sync.dma_start(out=xT[:, k, 1024:2048], in_=xr[:, k, 1024:2048])
                wk16 = load_w16(wkT_d)
                cosk = pcs.tile([128, T], BF, tag="tbl", name="cosk")
                nc.sync.dma_start(out=cosk[:, :], in_=cosk_d.ap())
                sink = pcs.tile([128, T], BF, tag="tbl", name="sink")
                nc.sync.dma_start(out=sink[:, :], in_=sink_d.ap())

                # sub-groups of (head, chunk) units; SG0 only touches token
                # chunks 0-1 so it can start on the first x half-rows.
                SGS = [
                    [(0, 0), (1, 0), (2, 0), (3, 0), (0, 1), (1, 1)],
                    [(2, 1), (3, 1), (0, 2), (1, 2), (2, 2), (3, 2)],
                    [(0, 3), (1, 3), (2, 3), (3, 3)],
                ]

                def evict_early_unit(a, h, c, cosT, sinT):
                    # reads the PSUM acc (frees its bank when done): square
                    # for the RMS stat + the two RoPE products
                    sq = psq.tile([128, F], BF, tag="sq")
                    nc.scalar.activation(
                        sq[:, :], a[:, :], mybir.ActivationFunctionType.Square,
                    )
                    rc = prc.tile([128, F], BF, tag="rc")
                    nc.vector.tensor_mul(rc[:, :], a[:, :], cosT[:, ts(c, 512)])
                    rot = prc.tile([128, F], BF, tag="rot")
                    nc.vector.tensor_mul(
                        rot[0:64, :], a[64:128, :], sinT[0:64, ts(c, 512)]
                    )
                    nc.vector.tensor_mul(
                        rot[64:128, :], a[0:64, :], sinT[64:128, ts(c, 512)]
                    )
                    return (sq, rc, rot)

                def evict_late(unit, sg, early, targets):
                    sq, rc, rot = early[unit]
                    h, c = sg[unit]
                    msq = psM.tile([128, F], F32, tag="msq")
                    nc.tensor.matmul(
                        msq[:, :], ones128[:, :], sq[:, :], start=True, stop=True
                    )
                    # rstd = (msq/HD + eps)^-0.5 via ln/exp on the ACT engine
                    # (same natural_log_exp table set as phase B's softmax
                    # exp; DVE reciprocal is ~8 cycles/elem -- way too slow)
                    lg = prstd.tile([128, F], F32, tag="lg")
                    nc.scalar.activation(
                        lg[:, :], msq[:, :], mybir.ActivationFunctionType.Ln,
                        bias=eps_t[:, :], scale=1.0 / HD,
                    )
                    rstd = prstd.tile([128, F], F32, tag="rstd")
                    nc.scalar.activation(
                        rstd[:, :], lg[:, :], mybir.ActivationFunctionType.Exp,
                        scale=-0.5,
                    )
                    summ = prc.tile([128, F], BF, tag="summ", bufs=2)
                    nc.vector.tensor_add(summ[:, :], rc[:, :], rot[:, :])
                    nc.vector.tensor_mul(
                        targets[h][:, ts(c, 512)], summ[:, :], rstd[:, :]
                    )

                def qk_phase(w16, targets, cosT, sinT, carry):
                    # carry: (early, sg, targets) of the previous phase's
                    # last SG -- its late evictions interleave with our
                    # first SG's k-loop so the PE never drains
                    pend = carry
                    for sg in SGS:
                        accs = [
                            psA.tile([128, F], F32, tag="acc", name="acc")
                            for i in range(len(sg))
                        ]
                        for k in range(TT):
                            for i, (a, (h, c)) in enumerate(zip(accs, sg)):
                                nc.tensor.matmul(
                                    a[:, :], w16[k][:, ts(h, 128)],
                                    xT[:, k, ts(c, 512)],
                                    start=(k == 0), stop=(k == TT - 1),
                                )
                                # stagger the previous SG's early evictions
                                # right behind this SG's last k-iteration so
                                # its PSUM banks free one by one
                                if k == TT - 1:
                                    early, esg = pend_early
                                    if i < len(esg):
                                        early.append(evict_early_unit(
                                            accs[i], esg[i][0], esg[i][1],
                                            cosT, sinT))
                            if k == 0:
                                pend_early = ([], sg)
                            # previous SG's (or phase's) late evictions: 2
                            # per k keeps the 2 msq PSUM banks rotating
                            # without stalling the PE
                            if pend is not None and k >= 1 and 2 * (k - 1) < len(pend[1]):
                                for u in (2 * (k - 1), 2 * (k - 1) + 1):
                                    if u < len(pend[1]):
                                        evict_late(u, pend[1], pend[0], pend[2])
                        pend = (pend_early[0], sg, targets)
                    return pend

                pend = qk_phase(wq16, QT, cosq, sinq, None)
                pend = qk_phase(wk16, KT, cosk, sink, pend)

                # -- V / G: x-stationary, token-partition output (unchanged
                #    math, per-k weight tiles) --
                def matmul_proj(acc, w16, t):
                    for k in range(TT):
                        nc.tensor.matmul(
                            acc[:, :], xT[:, k, ts(t, 128)], w16[k][:, :],
                            start=(k == 0), stop=(k == TT - 1),
                        )

                wv16 = load_w16(wvT_d)
                for t in range(TT):
                    acc = psA.tile([128, F], F32, tag="acc")
                    matmul_proj(acc, wv16, t)
                    # K's last sub-group finishes its normalization here,
                    # overlapped with the V matmul stream
                    if pend is not None and t >= 1 and 2 * (t - 1) < len(pend[1]):
                        for u in (2 * (t - 1), 2 * (t - 1) + 1):
                            if u < len(pend[1]):
                                evict_late(u, pend[1], pend[0], pend[2])
                    nc.scalar.copy(
                        vaug[:, t, :, 0:128],
                        acc[:, :].rearrange("p (h d) -> p h d", h=NHL),
                    )

                wg16 = load_w16(wgT_d)
                for t in range(TT):
                    acc = psA.tile([128, F], F32, tag="acc")
                    matmul_proj(acc, wg16, t)
                    nc.scalar.activation(
                        gs[:, t, :], acc[:, :], mybir.ActivationFunctionType.Silu
                    )

            # yun / woT pools are entered after phase A's pools release so
            # their SBUF space overlaps the (freed) x / weight staging area
            ypool = outer.enter_context(tc.tile_pool(name="ypool", bufs=1))
            wopool = outer.enter_context(tc.tile_pool(name="wopool", bufs=1))
            yun = ypool.tile([128, TT, NHL, HD], BF, tag="yun")  # unnormalized y
            woT = wopool.tile([128, 4, H], BF, tag="woT")
            nc.sync.dma_start(
                out=woT[:, :, :],
                in_=woT_d.ap().rearrange("(k p) n -> p k n", p=128),
            )

            # ================= phase B: SDPA =================
            with ExitStack() as pb:
                pE = pb.enter_context(tc.tile_pool(name="pE", bufs=21))
                pys = pb.enter_context(tc.tile_pool(name="pys", bufs=8))
                pyscr = pb.enter_context(tc.tile_pool(name="pyscr", bufs=2))
                pnw = pb.enter_context(tc.tile_pool(name="pnw", bufs=2))
                psS = pb.enter_context(tc.tile_pool(name="psS", bufs=3, space="PSUM"))
                psY = pb.enter_context(tc.tile_pool(name="psY", bufs=2, space="PSUM"))

                def s_block(c, h):
                    # S^T tiles, two k-tiles per 2-bank PSUM tile so the exp
                    # runs as one wide ACTIVATE (halves the per-op overhead)
                    elist = []
                    for kt0 in range(0, 4 * c + 4, 2):
                        e2 = pE.tile([128, 2, 512], BF, tag="e")
                        s2 = psS.tile([128, 2, 512], F32, tag="s")
                        diag = kt0 >= 4 * c
                        for j in range(2):
                            kt = kt0 + j
                            lo = 0 if kt < 4 * c else 128 * (kt - 4 * c)
                            nc.tensor.matmul(
                                s2[:, j, lo:512], KT[h][:, ts(kt, 128)],
                                QT[h][:, 512 * c + lo:512 * c + 512],
                                start=True, stop=True,
                            )
                        if not diag:
                            nc.scalar.activation(
                                e2[:, :, :], s2[:, :, :],
                                mybir.ActivationFunctionType.Exp, scale=SCALE,
                            )
                        else:
                            for j in range(2):
                                d = kt0 + j - 4 * c
                                lo = 128 * d
                                nc.scalar.activation(
                                    e2[:, j, lo:512], s2[:, j, lo:512],
                                    mybir.ActivationFunctionType.Exp, scale=SCALE,
                                )
                                nc.vector.tensor_mul(
                                    e2[:, j, lo:lo + 128], e2[:, j, lo:lo + 128],
                                    tri[:, :],
                                )
                        elist.append(e2)
                    return elist

                def pv_block(c, h, elist):
                    # PV + denominators, two query tiles per PSUM tile so the
                    # stats run batched on the DVE
                    for p in range(2):
                        y_ps = psY.tile([128, 2, 132], F32, tag="y")
                        for qp in range(2):
                            qt = 2 * p + qp
                            t = 4 * c + qt
                            for kt in range(t + 1):
                                nc.tensor.matmul(
                                    y_ps[:, qp, 0:129],
                                    elist[kt // 2][:, kt % 2, ts(qt, 128)],
                                    vaug[:, kt, h, 0:129],
                                    start=(kt == 0), stop=(kt == t),
                                )
                        tpair = 4 * c + 2 * p
                        # evict unnormalized y, then m = sumsq/HD + eps*s^2
                        nc.vector.tensor_copy(
                            yun[:, tpair:tpair + 2, h, :], y_ps[:, :, 0:128]
                        )
                        scol = pys.tile([128, 2], F32, tag="scol")
                        nc.vector.tensor_copy(scol[:, :], y_ps[:, :, 128])
                        s2e = pys.tile([128, 2], F32, tag="s2e")
                        nc.vector.tensor_mul(s2e[:, :], scol[:, :], scol[:, :])
                        ysq = pyscr.tile([128, 2, HD], BF, tag="ysq")
                        nc.vector.tensor_mul(
                            ysq[:, :, :], yun[:, tpair:tpair + 2, h, :],
                            yun[:, tpair:tpair + 2, h, :],
                        )
                        ss = pys.tile([128, 2], F32, tag="ss")
                        nc.vector.tensor_reduce(
                            out=ss[:, :], in_=ysq[:, :, :],
                            axis=mybir.AxisListType.X, op=mybir.AluOpType.add,
                        )
                        sse = pys.tile([128, 2], F32, tag="sse")
                        nc.vector.tensor_scalar(
                            out=sse[:, :], in0=ss[:, :], scalar1=1.0 / HD,
                            scalar2=None, op0=mybir.AluOpType.mult,
                        )
                        s2es = pys.tile([128, 2], F32, tag="s2es")
                        nc.vector.tensor_scalar(
                            out=s2es[:, :], in0=s2e[:, :], scalar1=EPS,
                            scalar2=None, op0=mybir.AluOpType.mult,
                        )
                        nc.vector.tensor_add(
                            m_all[:, tpair:tpair + 2, h], s2es[:, :], sse[:, :],
                        )

                def stok_chunk(c):
                    # stok = 1/sqrt(m) via bit-trick + 2 Newton steps, all on
                    # the DVE so the (ACT-bound) exp stream is untouched
                    y = stok[:, 4 * c:4 * c + 4, :]
                    mm = m_all[:, 4 * c:4 * c + 4, :]
                    hm = pnw.tile([128, 4, NHL], F32, tag="hm")
                    aa = pnw.tile([128, 4, NHL], F32, tag="aa")
                    cc = pnw.tile([128, 4, NHL], F32, tag="cc")
                    yi = y.bitcast(mybir.dt.int32)
                    nc.vector.tensor_scalar(
                        out=yi, in0=mm.bitcast(mybir.dt.int32), scalar1=1,
                        scalar2=None, op0=AL.logical_shift_right)
                    nc.vector.tensor_scalar(
                        out=yi, in0=yi, scalar1=0x5F3759DF, scalar2=-1,
                        op0=AL.subtract, op1=AL.mult)
                    nc.vector.tensor_scalar(
                        out=hm[:, :, :], in0=mm, scalar1=0.5, scalar2=None,
                        op0=AL.mult)
                    for _ in range(2):
                        nc.vector.tensor_mul(aa[:, :, :], y, y)
                        nc.vector.tensor_mul(aa[:, :, :], aa[:, :, :], hm[:, :, :])
                        nc.vector.tensor_scalar(
                            out=cc[:, :, :], in0=aa[:, :, :], scalar1=-1.0,
                            scalar2=1.5, op0=AL.mult, op1=AL.add)
                        nc.vector.tensor_mul(y, y, cc[:, :, :])

                units = [(c, h) for c in range(CH) for h in range(NHL)]
                prev = None
                for (c, h) in units:
                    el = s_block(c, h)
                    if prev is not None:
                        pv_block(*prev)
                        if prev[1] == NHL - 1:
                            stok_chunk(prev[0])
                    prev = (c, h, el)
                pv_block(*prev)
                stok_chunk(prev[0])

            # ================= phase C: normalize, gate, c_proj =================
            with ExitStack() as pc:
                pyg = pc.enter_context(tc.tile_pool(name="pyg", bufs=10))
                pygT = pc.enter_context(tc.tile_pool(name="pygT", bufs=16))
                pout = pc.enter_context(tc.tile_pool(name="pout", bufs=6))
                psTc = pc.enter_context(tc.tile_pool(name="psTc", bufs=4, space="PSUM"))
                psO = pc.enter_context(tc.tile_pool(name="psO", bufs=4, space="PSUM"))

                def ygm(t):
                    out = []
                    for hh in range(NHL):
                        t1 = pyg.tile([128, HD], BF, tag="t1")
                        nc.vector.tensor_mul(
                            t1[:, :], yun[:, t, hh, :], gs[:, t, ts(hh, 128)]
                        )
                        yg = pyg.tile([128, HD], BF, tag="yg", name=f"yg_{t}_{hh}")
                        if hh % 2 == 0:
                            nc.scalar.mul(yg[:, :], t1[:, :], stok[:, t, hh:hh + 1])
                        else:
                            nc.vector.tensor_scalar_mul(
                                yg[:, :], t1[:, :], stok[:, t, hh:hh + 1]
                            )
                        out.append(yg)
                    return out

                def emit_T(ygl):
                    out = []
                    for hh in range(NHL):
                        tp = psTc.tile([128, 128], BF, tag="tp")
                        nc.tensor.transpose(tp[:, :], ygl[hh][:, :], id128[:, :])
                        yt = pygT.tile([128, 128], BF, tag="yt")
                        if hh % 2 == 0:
                            nc.scalar.copy(yt[:, :], tp[:, :])
                        else:
                            nc.vector.tensor_copy(yt[:, :], tp[:, :])
                        out.append(yt)
                    return out

                pendT = [emit_T(ygm(0)), emit_T(ygm(1))]
                for t in range(TT):
                    ygT = pendT.pop(0)
                    if t + 2 < TT:
                        pendT.append(emit_T(ygm(t + 2)))
                    for n in range(4):
                        o_ps = psO.tile([128, 512], F32, tag="o")
                        for f in range(4):
                            nc.tensor.matmul(
                                o_ps[:, :], ygT[f][:, :], woT[:, f, ts(n, 512)],
                                start=(f == 0), stop=(f == 3),
                            )
                        o_sb = pout.tile([128, 512], BF, tag="osb")
                        if n % 2 == 0:
                            nc.vector.tensor_copy(o_sb[:, :], o_ps[:, :])
                        else:
                            nc.scalar.copy(o_sb[:, :], o_ps[:, :])
                        nc.sync.dma_start(
                            out=out_d.ap()[ts(t, 128), ts(n, 512)], in_=o_sb[:, :]
                        )

    nc.compile()
    return nc


def _rope_tables():
    inv_freq = 1.0 / (ROPE_BASE ** (np.arange(0, HD, 2, dtype=np.float32) / HD))
    t = np.arange(T, dtype=np.float32)
    freqs = t[:, None] * inv_freq[None, :]
    emb = np.concatenate([freqs, freqs], axis=-1)
    return np.cos(emb).astype(np.float32), np.sin(emb).astype(np.float32)


def _host_prep(x, Wq, Wk, Wv, Wg, Wo, q_gamma, k_gamma, o_gamma):
    x = np.asarray(x, dtype=np.float32)
    Wq = np.asarray(Wq, dtype=np.float32)
    Wk = np.asarray(Wk, dtype=np.float32)
    Wv = np.asarray(Wv, dtype=np.float32)
    Wg = np.asarray(Wg, dtype=np.float32)
    Wo = np.asarray(Wo, dtype=np.float32)
    q_gamma = np.asarray(q_gamma, dtype=np.float32)
    k_gamma = np.asarray(k_gamma, dtype=np.float32)
    o_gamma = np.asarray(o_gamma, dtype=np.float32)

    cos, sin = _rope_tables()
    sinm_f = np.concatenate([-sin[:, :64], sin[:, 64:]], axis=1)
    # (head_dim, token) layout tables; q_gamma*k_gamma folds into K's tables
    gqk = (q_gamma * k_gamma).astype(np.float32)
    cosqT = np.ascontiguousarray(cos.T).astype(BF16)
    sinqT = np.ascontiguousarray(sinm_f.T).astype(BF16)
    coskT = np.ascontiguousarray((cos * gqk[None, :]).T).astype(BF16)
    sinkT = np.ascontiguousarray((sinm_f * gqk[None, :]).T).astype(BF16)
    tri = (np.arange(128)[None, :] >= np.arange(128)[:, None]).astype(BF16)

    xTb = [np.ascontiguousarray(x[b].T).astype(BF16) for b in range(B)]
    per_group = []
    for g in range(4):
        hs = slice(g * F, (g + 1) * F)
        wo_scaled = Wo[:, hs] * np.tile(o_gamma, NHL)[None, :]
        per_group.append({
            "wqT": np.ascontiguousarray(Wq[hs].T).astype(BF16),
            "wkT": np.ascontiguousarray(Wk[hs].T).astype(BF16),
            "wvT": np.ascontiguousarray(Wv[hs].T).astype(BF16),
            "wgT": np.ascontiguousarray(Wg[hs].T).astype(BF16),
            "woT": np.ascontiguousarray(wo_scaled.T).astype(BF16),
        })

    in_maps = []
    for c in range(8):
        b, g = c // 4, c % 4
        m = {"xT": xTb[b], "cosqT": cosqT, "sinqT": sinqT, "coskT": coskT,
             "sinkT": sinkT, "tri": tri}
        m.update(per_group[g])
        in_maps.append(m)
    return in_maps


def kernel(x, Wq, Wk, Wv, Wg, Wo, q_gamma, k_gamma, o_gamma):
    if "nc" not in _CACHE:
        _CACHE["nc"] = _build_nc()
    nc = _CACHE["nc"]
    in_maps = _host_prep(x, Wq, Wk, Wv, Wg, Wo, q_gamma, k_gamma, o_gamma)
    res = bass_utils.run_bass_kernel_spmd(nc, in_maps, core_ids=list(range(8)))
    out = np.empty((B, T, H), dtype=np.float32)
    for b in range(B):
        acc = res.results[4 * b]["out"].astype(np.float32)
        for g in range(1, 4):
            acc = acc + res.results[4 * b + g]["out"].astype(np.float32)
        out[b] = acc
    return out

